# revision 3
# baseline (speedup 1.0000x reference)
"""Trainium2 Bass kernel for causal multi-head attention (B=4, S=2048, D=1024, H=16).

Sharding: 8 cores = (batch b in 0..3) x (head-group g in 0..1, 8 heads each).
Each core computes, for its (b, g):
  Q/K/V projections (local 512 dims), causal attention for 8 heads, and a
  partial output projection over its 512 head-dims. Host sums the two
  head-group partials per batch and adds the output bias.

v2 (vs the 512-chunk baseline): the Scalar/ACT engine was the measured
bottleneck (91% busy: 320 exps + DMAs + sem waits), with the PE at ~83%.
Changes:
  - q-chunks widened to 1024: scores for one (head, k-tile) land in a
    2-bank [128,1024] fp32 PSUM tile, giving ONE merged exp per (h, kti)
    (192 ACTIVATEs instead of 320, amortizing the ~175ns/instr ACT bubble).
  - heads processed singly (trace shows row-tiled score pairs do NOT
    overlap on HW, so pairing bought nothing) which frees PSUM banks:
    scores 2x2 + AV 1x2 + projection accum 2x1 = exactly 8 banks.
  - the Scalar queue carries zero DMAs; loads spread over sync/vector/
    gpsimd with (x-slab, weight-chunk) interleaving so the first
    projection matmuls start ~1.5us in.
  - everything bf16 (x, Wq/k/v, Wo; fp32 PSUM accumulation), halving
    input DMA bytes and enabling FWL fast weight loads.
  - prolog computes V + K[t0] + Q[t0] only (~20us instead of the full
    41us chunk-0 projection); the rest weaves into chunk-0 attention.
Attention orientation and normalization (ones-column denominator in the
A@V stationary operand, gpsimd partition_broadcast reciprocal) as before.
"""
import sys
import numpy as np

try:
    import concourse.bass as bass  # noqa: F401
except ImportError:  # pragma: no cover
    sys.path.insert(0, "/opt/trn_rl_repo")

from contextlib import ExitStack

import concourse.bacc as bacc
import concourse.tile as tile
import concourse.mybir as mybir
import concourse.bass_utils as bass_utils

B, S, D, H = 4, 2048, 1024, 16
DK = D // H            # 64
G = 2                  # head groups (cores per batch)
HPG = H // G           # 8 heads per core
DG = HPG * DK          # 512 local head dims
NCORES = B * G         # 8
SC = 1024              # q-chunk width
NQC = S // SC          # 2
NKT = S // 128         # 16 k-tiles

F32 = mybir.dt.float32
BF16 = mybir.dt.bfloat16
AF = mybir.ActivationFunctionType

LAST_RESULT = None     # BassKernelResults of the most recent run (for test.py)
_prog = None


def _build():
    nc = bacc.Bacc("TRN2", target_bir_lowering=False, debug=False,
                   num_devices=NCORES)
    xT = nc.dram_tensor("xT", [D, S], BF16, kind="ExternalInput").ap()
    wq = nc.dram_tensor("wq", [D, DG], BF16, kind="ExternalInput").ap()
    wk = nc.dram_tensor("wk", [D, DG], BF16, kind="ExternalInput").ap()
    wv = nc.dram_tensor("wv", [D, DG], BF16, kind="ExternalInput").ap()
    wo = nc.dram_tensor("wo", [DG, D], BF16, kind="ExternalInput").ap()
    bq = nc.dram_tensor("bq", [DG, 1], F32, kind="ExternalInput").ap()
    bk = nc.dram_tensor("bk", [DG, 1], F32, kind="ExternalInput").ap()
    bv = nc.dram_tensor("bv", [1, DG], BF16, kind="ExternalInput").ap()
    ones = nc.dram_tensor("ones", [1, 128], BF16, kind="ExternalInput").ap()
    vones = nc.dram_tensor("vones", [128, HPG, 1], BF16, kind="ExternalInput").ap()
    tri = nc.dram_tensor("tri", [128, 128], BF16, kind="ExternalInput").ap()
    out = nc.dram_tensor("out", [S, D], F32, kind="ExternalOutput").ap()

    with tile.TileContext(nc) as tc, ExitStack() as ctx:
        cpool = ctx.enter_context(tc.tile_pool(name="consts", bufs=1))
        qkpool = ctx.enter_context(tc.tile_pool(name="qk", bufs=1))
        vpool = ctx.enter_context(tc.tile_pool(name="vaug", bufs=1))
        vtpool = ctx.enter_context(tc.tile_pool(name="vt", bufs=1))
        wpool = ctx.enter_context(tc.tile_pool(name="wqkv", bufs=1))
        xpool = ctx.enter_context(tc.tile_pool(name="xs", bufs=4))
        atpool = ctx.enter_context(tc.tile_pool(name="at", bufs=4))
        pdpool = ctx.enter_context(tc.tile_pool(name="pd", bufs=2))
        scpool = ctx.enter_context(tc.tile_pool(name="scr", bufs=2))
        rpool = ctx.enter_context(tc.tile_pool(name="r0", bufs=4))
        tmpool = ctx.enter_context(tc.tile_pool(name="tmp", bufs=2))
        opool = ctx.enter_context(tc.tile_pool(name="ost", bufs=2))
        # PSUM: scores 2x[128,1024] (banks 0-3), AV 1x[128,1024] (4-5),
        # projection/oproj accumulators 2x[128,512] (6-7)
        ppsc = ctx.enter_context(tc.tile_pool(name="ppsc", bufs=2, space="PSUM"))
        ppav = ctx.enter_context(tc.tile_pool(name="ppav", bufs=1, space="PSUM"))
        ppacc = ctx.enter_context(tc.tile_pool(name="ppacc", bufs=2, space="PSUM"))

        xTr = xT.rearrange("(c p) s -> p c s", p=128)
        wqr = wq.rearrange("(c p) n -> p c n", p=128)
        wkr = wk.rearrange("(c p) n -> p c n", p=128)
        wvr = wv.rearrange("(c p) n -> p c n", p=128)

        # startup loads: (x-slab c, wq c) pairs so projection matmul c can
        # begin as soon as its pair lands; sync takes c 0..3, vector 4..7,
        # gpsimd takes wv + small constants. Scalar queue gets nothing.
        xs0_0 = xpool.tile([128, 4, SC], BF16, tag="xs", name="xs0_0")
        xs1_0 = xpool.tile([128, 4, SC], BF16, tag="xs", name="xs1_0")
        wq_t = wpool.tile([128, 8, DG], BF16)
        wk_t = wpool.tile([128, 8, DG], BF16)
        wv_t = wpool.tile([128, 8, DG], BF16)
        # scalar-queue DMAs here are startup-only: the ACT engine is idle
        # until the first exp (~20us in), and nothing is queued behind them
        for c in range(4):
            nc.sync.dma_start(xs0_0[:, c, :], xTr[:, c, 0:SC])
            nc.sync.dma_start(wq_t[:, c, :], wqr[:, c, :])
            nc.scalar.dma_start(xs1_0[:, c, :], xTr[:, 4 + c, 0:SC])
            nc.scalar.dma_start(wq_t[:, 4 + c, :], wqr[:, 4 + c, :])
        for c in range(8):
            nc.gpsimd.dma_start(wv_t[:, c, :], wvr[:, c, :])
        for c in range(4):
            nc.sync.dma_start(wk_t[:, c, :], wkr[:, c, :])
            nc.scalar.dma_start(wk_t[:, 4 + c, :], wkr[:, 4 + c, :])

        bq_t = cpool.tile([128, 4], F32)
        nc.gpsimd.dma_start(bq_t[:], bq.rearrange("(t p) o -> p (t o)", p=128))
        bk_t = cpool.tile([128, 4], F32)
        nc.gpsimd.dma_start(bk_t[:], bk.rearrange("(t p) o -> p (t o)", p=128))
        bv_t = cpool.tile([1, DG], BF16)
        nc.gpsimd.dma_start(bv_t[:], bv)
        ones_t = cpool.tile([1, 128], BF16)
        nc.gpsimd.dma_start(ones_t[:], ones)
        tri_t = cpool.tile([128, 128], BF16)
        nc.gpsimd.dma_start(tri_t[:], tri)

        qt = [qkpool.tile([128, S], BF16, name=f"qt{t}") for t in range(4)]
        kt_ = [qkpool.tile([128, S], BF16, name=f"kt{t}") for t in range(4)]
        vt = [vtpool.tile([128, S], BF16, name=f"vt{t}") for t in range(4)]
        va = [vpool.tile([128, HPG * 65], BF16, name=f"va{i}") for i in range(NKT)]
        for i in range(NKT):
            nc.gpsimd.dma_start(
                va[i].rearrange("p (h c) -> p h c", c=65)[:, :, 64:65], vones)
        # wo only needed by the output projection (chunk-1 filler onward)
        wo_t = cpool.tile([128, 4, D], BF16)
        nc.gpsimd.dma_start(wo_t[:], wo.rearrange("(c p) n -> p c n", p=128))

        def qgroup(sc, t, half, xc):
            pq = ppacc.tile([128, 512], F32, tag="pacc", name=f"pq{sc}_{t}_{half}")
            for c in range(8):
                nc.tensor.matmul(pq[:], wq_t[:, c, t * 128:(t + 1) * 128],
                                 xc(c)[:, half * 512:(half + 1) * 512],
                                 start=(c == 0), stop=(c == 7))
                yield
            lo = sc * SC + half * 512
            nc.vector.tensor_scalar_add(qt[t][:, lo:lo + 512], pq[:],
                                        bq_t[:, t:t + 1])

        def kgroup(sc, t, half, xc):
            pk = ppacc.tile([128, 512], F32, tag="pacc", name=f"pk{sc}_{t}_{half}")
            for c in range(8):
                nc.tensor.matmul(pk[:], wk_t[:, c, t * 128:(t + 1) * 128],
                                 xc(c)[:, half * 512:(half + 1) * 512],
                                 start=(c == 0), stop=(c == 7))
                yield
            lo = sc * SC + half * 512
            nc.vector.tensor_scalar_add(kt_[t][:, lo:lo + 512], pk[:],
                                        bk_t[:, t:t + 1])

        def vgroup(sc, ms, xc):
            pv = ppacc.tile([128, 512], F32, tag="pacc", name=f"pv{sc}_{ms}")
            for c in range(8):
                nc.tensor.matmul(pv[:], xc(c)[:, ms * 128:(ms + 1) * 128],
                                 wv_t[:, c, :], start=(c == 0), stop=False)
                yield
            nc.tensor.matmul(pv[:], ones_t[:], bv_t[:], start=False, stop=True)
            i = 8 * sc + ms
            nc.vector.tensor_copy(
                va[i].rearrange("p (h c) -> p h c", c=65)[:, :, 0:64],
                pv[:].rearrange("p (h c) -> p h c", c=64))
            yield

        def make_xc(sc, slabs=None):
            if slabs is not None:
                xs0, xs1 = slabs
            else:
                xs0 = xpool.tile([128, 4, SC], BF16, tag="xs", name=f"xs0_{sc}")
                xs1 = xpool.tile([128, 4, SC], BF16, tag="xs", name=f"xs1_{sc}")
                for c in range(4):
                    nc.sync.dma_start(xs0[:, c, :],
                                      xTr[:, c, sc * SC:(sc + 1) * SC])
                    nc.gpsimd.dma_start(xs1[:, c, :],
                                        xTr[:, 4 + c, sc * SC:(sc + 1) * SC])
            return lambda c: (xs0 if c < 4 else xs1)[:, c % 4, :]

        def proj_rest_gen(sc, xc):
            # Q/K for t=1..3 (t=0 and V are done in the prolog for sc=0)
            for t in range(1, 4):
                for half in range(2):
                    yield from kgroup(sc, t, half, xc)
                for half in range(2):
                    yield from qgroup(sc, t, half, xc)

        def proj_full_gen(sc):
            xc = make_xc(sc)
            for ms in range(8):
                yield from vgroup(sc, ms, xc)
            for half in range(2):
                yield from kgroup(sc, 0, half, xc)
            for half in range(2):
                yield from qgroup(sc, 0, half, xc)
            yield from proj_rest_gen(sc, xc)

        def oproj_g(m):
            ot = opool.tile([128, D], F32, tag="ost", name=f"ot{m}")
            for nh in range(2):
                pon = ppacc.tile([128, 512], F32, tag="pacc", name=f"pon{m}_{nh}")
                for t in range(4):
                    nc.tensor.matmul(pon[:], vt[t][:, m * 128:(m + 1) * 128],
                                     wo_t[:, t, nh * 512:(nh + 1) * 512],
                                     start=(t == 0), stop=(t == 3))
                    yield
                nc.vector.tensor_copy(ot[:, nh * 512:(nh + 1) * 512], pon[:])
                yield
            nc.sync.dma_start(out[m * 128:(m + 1) * 128, :], ot[:])

        def oproj_gen(ms):
            for m in ms:
                yield from oproj_g(m)

        def attn_head(h, qc, pull):
            t, po = h // 2, 64 * (h % 2)
            nkt = 8 * qc + 8
            pav = ppav.tile([128, SC], F32, tag="pav", name=f"pav{h}_{qc}")
            for kti in range(nkt):
                delta = max(kti * 128 - qc * SC, 0)
                pss = ppsc.tile([128, SC], F32, tag="ps", name=f"ps{h}_{qc}_{kti}")
                ks = kt_[t][po:po + 64, kti * 128:(kti + 1) * 128]
                q0 = qc * SC
                if delta < 512:
                    nc.tensor.matmul(pss[:, delta:512], ks,
                                     qt[t][po:po + 64, q0 + delta:q0 + 512],
                                     start=True, stop=True)
                h1lo = max(delta, 512)
                nc.tensor.matmul(pss[:, h1lo:SC], ks,
                                 qt[t][po:po + 64, q0 + h1lo:q0 + SC],
                                 start=True, stop=True)
                at = atpool.tile([128, SC], BF16, tag="at", name=f"at{h}_{qc}_{kti}")
                nc.scalar.activation(at[:, delta:SC], pss[:, delta:SC],
                                     AF.Exp, scale=0.125)
                if kti >= 8 * qc:  # diagonal 128x128 square of the band tile
                    sl = slice(delta, delta + 128)
                    nc.vector.tensor_mul(at[:, sl], at[:, sl], tri_t[:])
                vs = va[kti][:, h * 65:(h + 1) * 65]
                if delta < 512:
                    nc.tensor.matmul(pav[0:65, delta:512], vs, at[:, delta:512],
                                     start=(kti == 0), stop=(kti == 8 * qc + 3))
                nc.tensor.matmul(pav[0:65, h1lo:SC], vs, at[:, h1lo:SC],
                                 start=(kti == 0), stop=(kti == nkt - 1))
                pull(3 if qc == 0 else 2)
            # drain + normalize: denominator sits in row 64 of pav
            pd = pdpool.tile([128, SC], F32, tag="pd", name=f"pd{h}_{qc}")
            nc.vector.tensor_copy(pd[0:65, :], pav[0:65, :])
            rb = rpool.tile([128, 8], F32, tag="rb", name=f"rb{h}_{qc}")
            nc.gpsimd.dma_start(rb[:], pd[64:65, :])
            nc.vector.reciprocal(rb[:], rb[:])
            r0 = rpool.tile([1, SC], F32, tag="r0", name=f"r0{h}_{qc}")
            nc.gpsimd.dma_start(r0[:], rb[:])
            sct = scpool.tile([128, SC], F32, tag="scr", name=f"sc{h}_{qc}")
            nc.gpsimd.partition_broadcast(sct[0:64, :], r0[:])
            if po == 0:
                nc.vector.tensor_mul(vt[t][0:64, qc * SC:(qc + 1) * SC],
                                     pd[0:64, :], sct[0:64, :])
            else:
                tmp = tmpool.tile([64, SC], BF16, tag="tmp", name=f"tm{h}_{qc}")
                nc.vector.tensor_mul(tmp[:], pd[0:64, :], sct[0:64, :])
                nc.sync.dma_start(vt[t][64:128, qc * SC:(qc + 1) * SC], tmp[:])
            pull(2)

        # ---- prolog: chunk-0 V, K[t0], Q[t0] (minimum for attention h=0)
        xc0 = make_xc(0, slabs=(xs0_0, xs1_0))
        for ms in range(8):
            for _ in vgroup(0, ms, xc0):
                pass
        for half in range(2):
            for _ in kgroup(0, 0, half, xc0):
                pass
        for half in range(2):
            for _ in qgroup(0, 0, half, xc0):
                pass

        # ---- chunk 0: attention, weaving the rest of proj-0 then proj-1
        import itertools
        filler = itertools.chain(proj_rest_gen(0, xc0), proj_full_gen(1))

        def pull(n):
            for _ in range(n):
                if next(filler, "END") == "END":
                    return

        for h in range(HPG):
            attn_head(h, 0, pull)
        for _ in filler:   # any proj-1 leftovers before chunk-1 attention
            pass

        # ---- chunk 1: attention, weaving the first half of the output proj
        filler = oproj_gen(range(8))
        for h in range(HPG):
            attn_head(h, 1, pull)
        for _ in filler:
            pass
        # ---- tail: output projection for the chunk-1 sequence tiles
        for m in range(8, 16):
            for _ in oproj_g(m):
                pass

    nc.compile()
    return nc


def _program():
    global _prog
    if _prog is None:
        _prog = _build()
    return _prog


def kernel(x, mask, Wq, bq, Wk, bk, Wv, bv, Wo, bo):
    global LAST_RESULT
    import ml_dtypes
    x = np.asarray(x, dtype=np.float32)
    Wq = np.asarray(Wq, dtype=np.float32)
    Wk = np.asarray(Wk, dtype=np.float32)
    Wv = np.asarray(Wv, dtype=np.float32)
    Wo = np.asarray(Wo, dtype=np.float32)
    bq = np.asarray(bq, dtype=np.float32)
    bk = np.asarray(bk, dtype=np.float32)
    bv = np.asarray(bv, dtype=np.float32)
    bo = np.asarray(bo, dtype=np.float32)

    nc = _program()
    bf = ml_dtypes.bfloat16
    xTs = [np.ascontiguousarray(x[b].T).astype(bf) for b in range(B)]
    tri = np.zeros((128, 128), dtype=bf)
    tri[np.triu_indices(128)] = 1.0
    in_maps = []
    for c in range(NCORES):
        b, g = divmod(c, G)
        sl = slice(g * DG, (g + 1) * DG)
        in_maps.append({
            "xT": xTs[b],
            "wq": np.ascontiguousarray(Wq[sl, :].T).astype(bf),
            "wk": np.ascontiguousarray(Wk[sl, :].T).astype(bf),
            "wv": np.ascontiguousarray(Wv[sl, :].T).astype(bf),
            "wo": np.ascontiguousarray(Wo[:, sl].T).astype(bf),
            "bq": np.ascontiguousarray(bq[sl].reshape(DG, 1)),
            "bk": np.ascontiguousarray(bk[sl].reshape(DG, 1)),
            "bv": np.ascontiguousarray(bv[sl].reshape(1, DG)).astype(bf),
            "ones": np.ones((1, 128), dtype=bf),
            "vones": np.ones((128, HPG, 1), dtype=bf),
            "tri": tri,
        })
    res = bass_utils.run_bass_kernel_spmd(nc, in_maps,
                                          core_ids=list(range(NCORES)))
    LAST_RESULT = res
    outs = [r["out"] for r in res.results]
    y = np.stack([outs[G * b] + outs[G * b + 1] for b in range(B)])
    y += bo[None, None, :]
    return y.astype(np.float32)
